# revision 7
# baseline (speedup 1.0000x reference)
"""Trainium2 Bass kernel for nn_MicrofacetBase (Cook-Torrance microfacet base-class stub).

Computes, per sample i with rows light/normal/view in inputs[i]:
    hv    = light + view
    half  = hv / max(||hv||, EPS)
    nl    = normal.light ; nv = normal.view ; c = view.half
    fr    = cook-torrance fresnel(c, eta)   (branch via mask since eta is runtime)
    d     = 0 (MicrofacetBase stub)
    out   = base_color * (d * nl*nv * fr) / (4 * nl*nv)

Data-parallel across 8 NeuronCores: each core gets 500,000 samples (padded to
128*3912 = 500,736 so every SBUF tile is [128, *]).

Self-contained: hardcodes shapes/sharding; builds + runs the Bass program via
run_bass_kernel_spmd on cores 0-7.
"""

import math

import numpy as np

from concourse import bass, bacc, mybir
from concourse import tile
from concourse.bass_utils import run_bass_kernel_spmd

F32 = mybir.dt.float32
EPS = 1e-12

N_TOTAL = 4_000_000
N_CORES = 8
S = N_TOTAL // N_CORES          # samples per core = 500,000
ROWS = 3912                     # free-dim rows per partition (128*3912 = 500,736 >= S)
S_PAD = 128 * ROWS
TILE_ROWS = 489                 # rows per SBUF tile; 8 tiles per core
SQRT_HALF = 0.7071067811865476

Alu = mybir.AluOpType
Act = mybir.ActivationFunctionType


def build_program(eta: float, bc: np.ndarray, rows: int = ROWS, tile_rows: int = TILE_ROWS,
                  d_const: float = 0.0) -> bass.Bass:
    """Build the SPMD single-core program. eta/base_color are baked as immediates
    (JIT-style specialization on the scalar params)."""
    assert rows % tile_rows == 0
    n_tiles = rows // tile_rows
    # K = eta^2 - 1 in f32, matching e*e computed in f32
    K = float(np.float32(np.float32(eta) * np.float32(eta)) - np.float32(1.0))
    bc = [float(v) for v in np.asarray(bc, np.float32)]

    nc = bacc.Bacc(None)
    x = nc.declare_dram_parameter("x", [128, 9 * rows], F32, isOutput=False)
    y = nc.declare_dram_parameter("y", [128, 3 * rows], F32, isOutput=True)

    with tile.TileContext(nc) as tc:
        with tc.tile_pool(name="xp", bufs=2) as xp, \
             tc.tile_pool(name="pp", bufs=4) as pp, \
             tc.tile_pool(name="hp", bufs=2) as hp, \
             tc.tile_pool(name="op", bufs=2) as op_, \
             tc.tile_pool(name="sp", bufs=20) as sp:
            for i in range(n_tiles):
                Tm = tile_rows
                xt = xp.tile([128, 9 * Tm], F32, tag="xt", name=f"xt{i}")
                nc.sync.dma_start(out=xt[:], in_=x[:, bass.ts(i, 9 * Tm)])
                x3 = xt[:].rearrange("p (t c) -> p t c", c=9)
                li = x3[:, :, 0:3]
                nr = x3[:, :, 3:6]
                vw = x3[:, :, 6:9]

                def sc_tile(name):
                    return sp.tile([128, Tm], F32, tag="sc", name=f"{name}_{i}")

                # hv = light + view
                hv = hp.tile([128, 3 * Tm], F32, tag="hv", name=f"hv{i}")
                hv3 = hv[:].rearrange("p (t c) -> p t c", c=3)
                nc.vector.tensor_add(out=hv3, in0=li, in1=vw)

                def dot(pa, pb, name):
                    pr = pp.tile([128, 3 * Tm], F32, tag="pr", name=f"pr_{name}_{i}")
                    pr3 = pr[:].rearrange("p (t c) -> p t c", c=3)
                    nc.vector.tensor_mul(out=pr3, in0=pa, in1=pb)
                    d = sc_tile(name)
                    nc.vector.reduce_sum(out=d[:], in_=pr3, axis=mybir.AxisListType.X)
                    return d

                nl = dot(nr, li, "nl")     # normal . light
                nv = dot(nr, vw, "nv")     # normal . view
                dvh = dot(vw, hv3, "dvh")  # view . hv

                # s = hv . hv  (square on ACT to offload DVE)
                prs = pp.tile([128, 3 * Tm], F32, tag="pr", name=f"pr_s_{i}")
                nc.scalar.square(out=prs[:], in_=hv[:])
                s2 = sc_tile("s2")
                nc.vector.reduce_sum(
                    out=s2[:], in_=prs[:].rearrange("p (t c) -> p t c", c=3),
                    axis=mybir.AxisListType.X)

                # c = dvh / max(sqrt(s), EPS)
                nrm = sc_tile("nrm")
                nc.scalar.sqrt(out=nrm[:], in_=s2[:])
                nrmx = sc_tile("nrmx")
                nc.gpsimd.tensor_scalar_max(out=nrmx[:], in0=nrm[:], scalar1=EPS)
                inv = sc_tile("inv")
                nc.vector.reciprocal_approx_fast(out=inv[:], in_=nrmx[:])
                c = sc_tile("c")
                nc.vector.tensor_mul(out=c[:], in0=dvh[:], in1=inv[:])

                # fresnel
                c2 = sc_tile("c2")
                nc.scalar.square(out=c2[:], in_=c[:])
                gg = sc_tile("gg")
                nc.gpsimd.tensor_scalar_add(out=gg[:], in0=c2[:], scalar1=K)
                mask = sc_tile("mask")
                nc.gpsimd.tensor_scalar(out=mask[:], in0=gg[:], scalar1=0.0,
                                        scalar2=None, op0=Alu.is_gt)
                ggm = sc_tile("ggm")
                nc.gpsimd.tensor_scalar_max(out=ggm[:], in0=gg[:], scalar1=EPS)
                gs = sc_tile("gs")
                nc.scalar.sqrt(out=gs[:], in_=ggm[:])

                u = sc_tile("u")
                nc.vector.tensor_mul(out=u[:], in0=c[:], in1=gs[:])
                # b-num = c*(gs+c)-1 = (u - 1) + c2 ; b-den = c*(gs-c)+1 = (u + 1) - c2
                bnum = sc_tile("bnum")
                nc.vector.scalar_tensor_tensor(out=bnum[:], in0=u[:], scalar=-1.0,
                                               in1=c2[:], op0=Alu.add, op1=Alu.add)
                bden = sc_tile("bden")
                nc.vector.scalar_tensor_tensor(out=bden[:], in0=u[:], scalar=1.0,
                                               in1=c2[:], op0=Alu.add, op1=Alu.subtract)
                den1 = sc_tile("den1")
                nc.vector.tensor_add(out=den1[:], in0=gs[:], in1=c[:])
                num1 = sc_tile("num1")
                nc.vector.tensor_sub(out=num1[:], in0=gs[:], in1=c[:])
                r1 = sc_tile("r1")
                nc.vector.reciprocal_approx_fast(out=r1[:], in_=den1[:])
                a = sc_tile("a")
                nc.vector.tensor_mul(out=a[:], in0=num1[:], in1=r1[:])
                r2 = sc_tile("r2")
                nc.vector.reciprocal_approx_fast(out=r2[:], in_=bden[:])
                b = sc_tile("b")
                nc.vector.tensor_mul(out=b[:], in0=bnum[:], in1=r2[:])
                ab = sc_tile("ab")
                nc.vector.tensor_mul(out=ab[:], in0=a[:], in1=b[:])
                # fr_val = 0.5*a^2*(1+b^2) = (a/sqrt2)^2 + (ab/sqrt2)^2
                sa = sc_tile("sa")
                nc.scalar.activation(out=sa[:], in_=a[:], func=Act.Square, scale=SQRT_HALF)
                sab = sc_tile("sab")
                nc.scalar.activation(out=sab[:], in_=ab[:], func=Act.Square, scale=SQRT_HALF)
                frv = sc_tile("frv")
                nc.vector.tensor_add(out=frv[:], in0=sa[:], in1=sab[:])
                # fr = where(gg > 0, fr_val, 1.0) = (frv - 1)*mask + 1
                frm1 = sc_tile("frm1")
                nc.vector.scalar_tensor_tensor(out=frm1[:], in0=frv[:], scalar=-1.0,
                                               in1=mask[:], op0=Alu.add, op1=Alu.mult)
                fr = sc_tile("fr")
                nc.gpsimd.tensor_scalar_add(out=fr[:], in0=frm1[:], scalar1=1.0)

                # scale = d * (nl*nv) * fr / (4*nl*nv)
                g = sc_tile("g")
                nc.vector.tensor_mul(out=g[:], in0=nl[:], in1=nv[:])
                z1 = sc_tile("z1")
                nc.scalar.mul(out=z1[:], in_=g[:], mul=d_const)
                z2 = sc_tile("z2")
                nc.vector.tensor_mul(out=z2[:], in0=z1[:], in1=fr[:])
                dn = sc_tile("dn")
                nc.scalar.mul(out=dn[:], in_=g[:], mul=4.0)
                rdn = sc_tile("rdn")
                nc.vector.reciprocal_approx_fast(out=rdn[:], in_=dn[:])
                scl = sc_tile("scl")
                nc.vector.tensor_mul(out=scl[:], in0=z2[:], in1=rdn[:])

                # out[t, k] = base_color[k] * scale[t]
                ot = op_.tile([128, 3 * Tm], F32, tag="ot", name=f"ot{i}")
                o3 = ot[:].rearrange("p (t c) -> p t c", c=3)
                for k in range(3):
                    nc.scalar.mul(out=o3[:, :, k], in_=scl[:], mul=bc[k])
                nc.sync.dma_start(out=y[:, bass.ts(i, 3 * Tm)], in_=ot[:])
    if not nc.is_finalized():
        nc.finalize()
    return nc


def _shard_inputs(inputs: np.ndarray) -> list:
    flat = np.ascontiguousarray(inputs, dtype=np.float32).reshape(N_TOTAL, 9)
    pad = np.ones((S_PAD - S, 9), dtype=np.float32)
    in_maps = []
    for cidx in range(N_CORES):
        sh = np.concatenate([flat[cidx * S:(cidx + 1) * S], pad], axis=0)
        in_maps.append({"x": sh.reshape(128, 9 * ROWS)})
    return in_maps


def _assemble(results: list) -> np.ndarray:
    outs = []
    for cidx in range(N_CORES):
        o = np.asarray(results[cidx]["y"], dtype=np.float32)
        outs.append(o.reshape(S_PAD, 3)[:S])
    return np.concatenate(outs, axis=0)


def run(inputs, base_color, alpha, eta, trace=False, **trace_kwargs):
    del alpha  # unused by MicrofacetBase (d-term stub)
    nc = build_program(float(np.asarray(eta).reshape(-1)[0]),
                       np.asarray(base_color, np.float32).reshape(3))
    in_maps = _shard_inputs(np.asarray(inputs))
    res = run_bass_kernel_spmd(nc, in_maps, list(range(N_CORES)), trace=trace,
                               **trace_kwargs)
    out = _assemble(res.results)
    return out, res


def kernel(inputs, base_color, alpha, eta):
    out, _ = run(inputs, base_color, alpha, eta, trace=False)
    return out


# revision 10
# speedup vs baseline: 2.1291x; 2.1291x over previous
"""Trainium2 Bass kernel for nn_MicrofacetBase (Cook-Torrance microfacet base-class stub).

Computes, per sample i with rows light/normal/view in inputs[i]:
    hv    = light + view
    half  = hv / max(||hv||, EPS)
    nl    = normal.light ; nv = normal.view ; c = view.half
    fr    = cook-torrance fresnel(c, eta)   (branch via mask since eta is runtime)
    d     = 0 (MicrofacetBase stub)
    out   = base_color * (d * nl*nv * fr) / (4 * nl*nv)

Data-parallel across 8 NeuronCores: each core gets 500,000 samples (padded to
128*3912 = 500,736 so every SBUF tile is [128, *]).

Self-contained: hardcodes shapes/sharding; builds + runs the Bass program via
run_bass_kernel_spmd on cores 0-7.
"""

import math

import numpy as np

from concourse import bass, bacc, mybir
from concourse import tile
from concourse.bass_utils import run_bass_kernel_spmd

F32 = mybir.dt.float32
EPS = 1e-12

N_TOTAL = 4_000_000
N_CORES = 8
S = N_TOTAL // N_CORES          # samples per core = 500,000
ROWS = 3912                     # free-dim rows per partition (128*3912 = 500,736 >= S)
S_PAD = 128 * ROWS
TILE_ROWS = 489                 # rows per SBUF tile; 8 tiles per core
SQRT_HALF = 0.7071067811865476

Alu = mybir.AluOpType
Act = mybir.ActivationFunctionType


def build_program(eta: float, bc: np.ndarray, rows: int = ROWS, tile_rows: int = TILE_ROWS,
                  d_const: float = 0.0) -> bass.Bass:
    """Build the SPMD single-core program. eta/base_color are baked as immediates
    (JIT-style specialization on the scalar params)."""
    assert rows % tile_rows == 0
    n_tiles = rows // tile_rows
    # K = eta^2 - 1 in f32, matching e*e computed in f32
    K = float(np.float32(np.float32(eta) * np.float32(eta)) - np.float32(1.0))
    bc = [float(v) for v in np.asarray(bc, np.float32)]

    nc = bacc.Bacc(None)
    if K > 1e-6:
        # register eta^2-1 as a const AP so Act.Sqrt can take it as bias
        kt = nc.alloc_sbuf_tensor(f"const-K", [128, 1], F32)
        nc.gpsimd.memset(kt.ap(), K)
        nc.const_aps.aps[(F32, float(K))] = kt.ap()
        nc.all_engine_barrier()
    x = nc.declare_dram_parameter("x", [128, 9 * rows], F32, isOutput=False)
    y = nc.declare_dram_parameter("y", [128, 3 * rows], F32, isOutput=True)

    with tile.TileContext(nc) as tc:
        with tc.tile_pool(name="xp", bufs=2) as xp, \
             tc.tile_pool(name="pp", bufs=4) as pp, \
             tc.tile_pool(name="hp", bufs=2) as hp, \
             tc.tile_pool(name="op", bufs=2) as op_, \
             tc.tile_pool(name="sp", bufs=20) as sp:
            for i in range(n_tiles):
                Tm = tile_rows
                xt = xp.tile([128, 9 * Tm], F32, tag="xt", name=f"xt{i}")
                nc.sync.dma_start(out=xt[:], in_=x[:, bass.ts(i, 9 * Tm)])
                x3 = xt[:].rearrange("p (t c) -> p t c", c=9)
                li = x3[:, :, 0:3]
                nr = x3[:, :, 3:6]
                vw = x3[:, :, 6:9]

                def sc_tile(name):
                    return sp.tile([128, Tm], F32, tag="sc", name=f"{name}_{i}")

                # hv = light + view
                hv = hp.tile([128, 3 * Tm], F32, tag="hv", name=f"hv{i}")
                hv3 = hv[:].rearrange("p (t c) -> p t c", c=3)
                nc.vector.tensor_add(out=hv3, in0=li, in1=vw)

                def dot(pa, pb, name):
                    pr = pp.tile([128, 3 * Tm], F32, tag="pr", name=f"pr_{name}_{i}")
                    pr3 = pr[:].rearrange("p (t c) -> p t c", c=3)
                    nc.vector.tensor_mul(out=pr3, in0=pa, in1=pb)
                    d = sc_tile(name)
                    nc.vector.reduce_sum(out=d[:], in_=pr3, axis=mybir.AxisListType.X)
                    return d

                nl = dot(nr, li, "nl")     # normal . light
                nv = dot(nr, vw, "nv")     # normal . view
                dvh = dot(vw, hv3, "dvh")  # view . hv

                # s = hv . hv  (square on ACT to offload DVE)
                prs = pp.tile([128, 3 * Tm], F32, tag="pr", name=f"pr_s_{i}")
                nc.scalar.square(out=prs[:], in_=hv[:])
                s2 = sc_tile("s2")
                nc.vector.reduce_sum(
                    out=s2[:], in_=prs[:].rearrange("p (t c) -> p t c", c=3),
                    axis=mybir.AxisListType.X)

                # c = dvh / max(sqrt(s), EPS)
                nrm = sc_tile("nrm")
                nc.scalar.sqrt(out=nrm[:], in_=s2[:])
                nrmx = sc_tile("nrmx")
                nc.vector.tensor_scalar_max(out=nrmx[:], in0=nrm[:], scalar1=EPS)
                inv = sc_tile("inv")
                nc.vector.reciprocal_approx_fast(out=inv[:], in_=nrmx[:])
                c = sc_tile("c")
                nc.vector.tensor_mul(out=c[:], in0=dvh[:], in1=inv[:])

                # fresnel: gg = c^2 + K with K = eta^2-1
                c2 = sc_tile("c2")
                nc.scalar.square(out=c2[:], in_=c[:])
                always_pos = K > 1e-6  # gg = c^2 + K >= K > 0: branch statically true
                if always_pos:
                    # gs = sqrt(gg) fused: Sqrt(c2 + K); max(gg,EPS) is a no-op
                    gs = sc_tile("gs")
                    nc.scalar.activation(out=gs[:], in_=c2[:], func=Act.Sqrt, bias=K)
                else:
                    gg = sc_tile("gg")
                    nc.vector.tensor_scalar_add(out=gg[:], in0=c2[:], scalar1=K)
                    mask = sc_tile("mask")
                    nc.vector.tensor_scalar(out=mask[:], in0=gg[:], scalar1=0.0,
                                            scalar2=None, op0=Alu.is_gt)
                    ggm = sc_tile("ggm")
                    nc.vector.tensor_scalar_max(out=ggm[:], in0=gg[:], scalar1=EPS)
                    gs = sc_tile("gs")
                    nc.scalar.sqrt(out=gs[:], in_=ggm[:])

                u = sc_tile("u")
                nc.vector.tensor_mul(out=u[:], in0=c[:], in1=gs[:])
                # b-num = c*(gs+c)-1 = (u - 1) + c2 ; b-den = c*(gs-c)+1 = (u + 1) - c2
                bnum = sc_tile("bnum")
                nc.vector.scalar_tensor_tensor(out=bnum[:], in0=u[:], scalar=-1.0,
                                               in1=c2[:], op0=Alu.add, op1=Alu.add)
                bden = sc_tile("bden")
                nc.vector.scalar_tensor_tensor(out=bden[:], in0=u[:], scalar=1.0,
                                               in1=c2[:], op0=Alu.add, op1=Alu.subtract)
                den1 = sc_tile("den1")
                nc.vector.tensor_add(out=den1[:], in0=gs[:], in1=c[:])
                num1 = sc_tile("num1")
                nc.vector.tensor_sub(out=num1[:], in0=gs[:], in1=c[:])
                r1 = sc_tile("r1")
                nc.vector.reciprocal_approx_fast(out=r1[:], in_=den1[:])
                a = sc_tile("a")
                nc.vector.tensor_mul(out=a[:], in0=num1[:], in1=r1[:])
                r2 = sc_tile("r2")
                nc.vector.reciprocal_approx_fast(out=r2[:], in_=bden[:])
                b = sc_tile("b")
                nc.vector.tensor_mul(out=b[:], in0=bnum[:], in1=r2[:])
                ab = sc_tile("ab")
                nc.vector.tensor_mul(out=ab[:], in0=a[:], in1=b[:])
                # fr_val = 0.5*a^2*(1+b^2) = (a/sqrt2)^2 + (ab/sqrt2)^2
                sa = sc_tile("sa")
                nc.scalar.activation(out=sa[:], in_=a[:], func=Act.Square, scale=SQRT_HALF)
                sab = sc_tile("sab")
                nc.scalar.activation(out=sab[:], in_=ab[:], func=Act.Square, scale=SQRT_HALF)
                frv = sc_tile("frv")
                nc.vector.tensor_add(out=frv[:], in0=sa[:], in1=sab[:])
                if always_pos:
                    fr = frv  # where(gg>0, frv, 1.0) with gg>0 statically true
                else:
                    # fr = where(gg > 0, fr_val, 1.0) = (frv - 1)*mask + 1
                    frm1 = sc_tile("frm1")
                    nc.vector.scalar_tensor_tensor(out=frm1[:], in0=frv[:], scalar=-1.0,
                                                   in1=mask[:], op0=Alu.add, op1=Alu.mult)
                    fr = sc_tile("fr")
                    nc.vector.tensor_scalar_add(out=fr[:], in0=frm1[:], scalar1=1.0)

                # scale = d * (nl*nv) * fr / (4*nl*nv)
                g = sc_tile("g")
                nc.vector.tensor_mul(out=g[:], in0=nl[:], in1=nv[:])
                z1 = sc_tile("z1")
                nc.scalar.mul(out=z1[:], in_=g[:], mul=d_const)
                z2 = sc_tile("z2")
                nc.vector.tensor_mul(out=z2[:], in0=z1[:], in1=fr[:])
                dn = sc_tile("dn")
                nc.scalar.mul(out=dn[:], in_=g[:], mul=4.0)
                rdn = sc_tile("rdn")
                nc.vector.reciprocal_approx_fast(out=rdn[:], in_=dn[:])
                scl = sc_tile("scl")
                nc.vector.tensor_mul(out=scl[:], in0=z2[:], in1=rdn[:])

                # out[t, k] = base_color[k] * scale[t]
                ot = op_.tile([128, 3 * Tm], F32, tag="ot", name=f"ot{i}")
                o3 = ot[:].rearrange("p (t c) -> p t c", c=3)
                for k in range(3):
                    nc.scalar.mul(out=o3[:, :, k], in_=scl[:], mul=bc[k])
                nc.sync.dma_start(out=y[:, bass.ts(i, 3 * Tm)], in_=ot[:])
    if not nc.is_finalized():
        nc.finalize()
    return nc


def _shard_inputs(inputs: np.ndarray) -> list:
    flat = np.ascontiguousarray(inputs, dtype=np.float32).reshape(N_TOTAL, 9)
    pad = np.ones((S_PAD - S, 9), dtype=np.float32)
    in_maps = []
    for cidx in range(N_CORES):
        sh = np.concatenate([flat[cidx * S:(cidx + 1) * S], pad], axis=0)
        in_maps.append({"x": sh.reshape(128, 9 * ROWS)})
    return in_maps


def _assemble(results: list) -> np.ndarray:
    outs = []
    for cidx in range(N_CORES):
        o = np.asarray(results[cidx]["y"], dtype=np.float32)
        outs.append(o.reshape(S_PAD, 3)[:S])
    return np.concatenate(outs, axis=0)


def run(inputs, base_color, alpha, eta, trace=False, **trace_kwargs):
    del alpha  # unused by MicrofacetBase (d-term stub)
    nc = build_program(float(np.asarray(eta).reshape(-1)[0]),
                       np.asarray(base_color, np.float32).reshape(3))
    in_maps = _shard_inputs(np.asarray(inputs))
    res = run_bass_kernel_spmd(nc, in_maps, list(range(N_CORES)), trace=trace,
                               **trace_kwargs)
    out = _assemble(res.results)
    return out, res


def kernel(inputs, base_color, alpha, eta):
    out, _ = run(inputs, base_color, alpha, eta, trace=False)
    return out


# revision 13
# speedup vs baseline: 2.1416x; 1.0058x over previous
"""Trainium2 Bass kernel for nn_MicrofacetBase (Cook-Torrance microfacet base-class stub).

Per sample i with rows light/normal/view in inputs[i]:
    hv    = light + view
    half  = hv / max(||hv||, EPS)
    nl    = normal.light ; nv = normal.view ; c = view.half
    fr    = cook-torrance fresnel(c, eta)
    d     = 0 (MicrofacetBase stub)
    out   = base_color * (d * nl*nv * fr) / (4 * nl*nv)

Data-parallel across 8 NeuronCores: each core gets 500,000 samples (padded to
128*3912 = 500,736 so every SBUF tile is [128, *]).

Layout: per core the input is reshaped to [128, ROWS, 9] (row-major) so each
partition owns ROWS consecutive samples. Component stage runs per tile of
T rows (3 products + hv written back into the input tile + 2 fused reduces,
products in bf16 for 2x reduce rate); the per-sample scalar chain is batched
over G=2 tiles (width W=2T) to amortize per-instruction overhead.

Self-contained: hardcodes shapes/sharding; builds + runs the Bass program via
run_bass_kernel_spmd on cores 0-7.
"""

import numpy as np

from concourse import bass, bacc, mybir
from concourse import tile
from concourse.bass_utils import run_bass_kernel_spmd

F32 = mybir.dt.float32
BF16 = mybir.dt.bfloat16
EPS = 1e-12

N_TOTAL = 4_000_000
N_CORES = 8
S = N_TOTAL // N_CORES          # samples per core = 500,000
ROWS = 3912                     # rows per partition (128*3912 = 500,736 >= S)
S_PAD = 128 * ROWS
TILE_ROWS = 652                 # rows per tile; 6 tiles per core
GROUP = 2                       # tiles per scalar-chain batch
SQRT_HALF = 0.7071067811865476

Alu = mybir.AluOpType
Act = mybir.ActivationFunctionType


def build_program(eta: float, bc: np.ndarray, rows: int = ROWS, tile_rows: int = TILE_ROWS,
                  group: int = GROUP, d_const: float = 0.0) -> bass.Bass:
    """Build the SPMD single-core program. eta/base_color are baked as immediates
    (JIT-style specialization on the scalar params)."""
    assert rows % tile_rows == 0
    n_tiles = rows // tile_rows
    assert n_tiles % group == 0
    n_groups = n_tiles // group
    T = tile_rows
    W = group * T
    K = float(np.float32(np.float32(eta) * np.float32(eta)) - np.float32(1.0))
    bc = [float(v) for v in np.asarray(bc, np.float32)]
    bc_uniform = bc[0] == bc[1] == bc[2]
    always_pos = K > 1e-6  # gg = c^2 + K >= K > 0: where-branch statically true

    nc = bacc.Bacc(None)
    if always_pos:
        # register eta^2-1 as a const AP so Act.Sqrt can take it as bias
        kt = nc.alloc_sbuf_tensor("const-K", [128, 1], F32)
        nc.gpsimd.memset(kt.ap(), K)
        nc.const_aps.aps[(F32, float(K))] = kt.ap()
        nc.all_engine_barrier()
    x = nc.declare_dram_parameter("x", [128, 9 * rows], F32, isOutput=False)
    y = nc.declare_dram_parameter("y", [128, 3 * rows], F32, isOutput=True)

    with tile.TileContext(nc) as tc:
        with tc.tile_pool(name="xp", bufs=2) as xp, \
             tc.tile_pool(name="pp", bufs=2) as pp, \
             tc.tile_pool(name="dp", bufs=2) as dp, \
             tc.tile_pool(name="op", bufs=2) as op_, \
             tc.tile_pool(name="sp", bufs=10) as sp:
            for gi in range(n_groups):
                # ---- component stage: per tile ----
                D = dp.tile([128, 4 * W], F32, tag="D", name=f"D{gi}")
                Dv = D[:].rearrange("p (s w) -> p s w", s=4)
                for tg in range(group):
                    i = gi * group + tg
                    xt = xp.tile([128, 9 * T], F32, tag="xt", name=f"xt{i}")
                    nc.sync.dma_start(out=xt[:], in_=x[:, bass.ts(i, 9 * T)])
                    x3 = xt[:].rearrange("p (t c) -> p t c", c=9)
                    li = x3[:, :, 0:3]
                    nr = x3[:, :, 3:6]
                    vw = x3[:, :, 6:9]

                    # products (bf16) for nl, nv into pr3 sections 0,1
                    pr3 = pp.tile([128, 9 * T], BF16, tag="pr", name=f"pr{i}")
                    p3v = pr3[:].rearrange("p (s t c) -> p s t c", s=3, c=3)
                    nc.vector.tensor_mul(out=p3v[:, 0], in0=nr, in1=li)
                    nc.vector.tensor_mul(out=p3v[:, 1], in0=nr, in1=vw)
                    # hv = light + view, written over the (now dead) normal slots
                    nc.vector.tensor_add(out=nr, in0=li, in1=vw)
                    hv = nr
                    nc.vector.tensor_mul(out=p3v[:, 2], in0=vw, in1=hv)
                    # s-products on ACT
                    prs = pp.tile([128, 3 * T], BF16, tag="prs", name=f"prs{i}")
                    nc.scalar.square(out=prs[:].rearrange("p (t c) -> p t c", c=3), in_=hv)

                    # fused reduces: [nl|nv|dvh] and [s2] into D at col tg*T
                    nc.vector.reduce_sum(
                        out=Dv[:, 0:3, bass.ts(tg, T)], in_=p3v,
                        axis=mybir.AxisListType.X)
                    nc.vector.reduce_sum(
                        out=Dv[:, 3, bass.ts(tg, T)],
                        in_=prs[:].rearrange("p (t c) -> p t c", c=3),
                        axis=mybir.AxisListType.X)

                # ---- per-sample scalar chain: width W ----
                nl = Dv[:, 0]
                nv = Dv[:, 1]
                dvh = Dv[:, 2]
                s2 = Dv[:, 3]

                def sc_tile(name):
                    return sp.tile([128, W], F32, tag="sc", name=f"{name}_{gi}")

                # c = dvh / max(sqrt(s), EPS)
                nrm = sc_tile("nrm")
                nc.scalar.sqrt(out=nrm[:], in_=s2)
                nrmx = sc_tile("nrmx")
                nc.vector.tensor_scalar_max(out=nrmx[:], in0=nrm[:], scalar1=EPS)
                inv = sc_tile("inv")
                nc.vector.reciprocal_approx_fast(out=inv[:], in_=nrmx[:])
                c = sc_tile("c")
                nc.vector.tensor_mul(out=c[:], in0=dvh, in1=inv[:])

                # fresnel: gg = c^2 + K
                c2 = sc_tile("c2")
                nc.scalar.square(out=c2[:], in_=c[:])
                if always_pos:
                    gs = sc_tile("gs")
                    nc.scalar.activation(out=gs[:], in_=c2[:], func=Act.Sqrt, bias=K)
                else:
                    gg = sc_tile("gg")
                    nc.vector.tensor_scalar_add(out=gg[:], in0=c2[:], scalar1=K)
                    mask = sc_tile("mask")
                    nc.vector.tensor_scalar(out=mask[:], in0=gg[:], scalar1=0.0,
                                            scalar2=None, op0=Alu.is_gt)
                    ggm = sc_tile("ggm")
                    nc.vector.tensor_scalar_max(out=ggm[:], in0=gg[:], scalar1=EPS)
                    gs = sc_tile("gs")
                    nc.scalar.sqrt(out=gs[:], in_=ggm[:])

                u = sc_tile("u")
                nc.vector.tensor_mul(out=u[:], in0=c[:], in1=gs[:])
                # b-num = c*(gs+c)-1 = (u-1)+c2 ; b-den = c*(gs-c)+1 = (u+1)-c2
                bnum = sc_tile("bnum")
                nc.vector.scalar_tensor_tensor(out=bnum[:], in0=u[:], scalar=-1.0,
                                               in1=c2[:], op0=Alu.add, op1=Alu.add)
                bden = sc_tile("bden")
                nc.vector.scalar_tensor_tensor(out=bden[:], in0=u[:], scalar=1.0,
                                               in1=c2[:], op0=Alu.add, op1=Alu.subtract)
                den1 = sc_tile("den1")
                nc.vector.tensor_add(out=den1[:], in0=gs[:], in1=c[:])
                num1 = sc_tile("num1")
                nc.vector.tensor_sub(out=num1[:], in0=gs[:], in1=c[:])
                def safe_recip(src, nm):
                    # sign(x) * recip(max(|x|, 1e-10)): recip_approx_fast is
                    # undefined at +-0, and bf16 product cancellation can make
                    # these denominators exactly 0 where f32 would not.
                    aa = sc_tile(nm + "_abs")
                    nc.scalar.activation(out=aa[:], in_=src[:], func=Act.Abs)
                    nc.vector.tensor_scalar_max(out=aa[:], in0=aa[:], scalar1=1e-10)
                    rm = sc_tile(nm + "_rm")
                    nc.vector.reciprocal_approx_fast(out=rm[:], in_=aa[:])
                    sg = sc_tile(nm + "_sg")
                    nc.scalar.sign(out=sg[:], in_=src[:])
                    rr = sc_tile(nm)
                    nc.vector.tensor_mul(out=rr[:], in0=rm[:], in1=sg[:])
                    return rr

                r1 = sc_tile("r1")
                nc.vector.reciprocal_approx_fast(out=r1[:], in_=den1[:])
                a = sc_tile("a")
                nc.vector.tensor_mul(out=a[:], in0=num1[:], in1=r1[:])
                r2 = safe_recip(bden, "r2")
                b = sc_tile("b")
                nc.vector.tensor_mul(out=b[:], in0=bnum[:], in1=r2[:])
                ab = sc_tile("ab")
                nc.vector.tensor_mul(out=ab[:], in0=a[:], in1=b[:])
                # fr_val = 0.5*a^2*(1+b^2) = (a/sqrt2)^2 + (ab/sqrt2)^2
                sa = sc_tile("sa")
                nc.scalar.activation(out=sa[:], in_=a[:], func=Act.Square, scale=SQRT_HALF)
                sab = sc_tile("sab")
                nc.scalar.activation(out=sab[:], in_=ab[:], func=Act.Square, scale=SQRT_HALF)
                frv = sc_tile("frv")
                nc.vector.tensor_add(out=frv[:], in0=sa[:], in1=sab[:])
                if always_pos:
                    fr = frv
                else:
                    frm1 = sc_tile("frm1")
                    nc.vector.scalar_tensor_tensor(out=frm1[:], in0=frv[:], scalar=-1.0,
                                                   in1=mask[:], op0=Alu.add, op1=Alu.mult)
                    fr = sc_tile("fr")
                    nc.vector.tensor_scalar_add(out=fr[:], in0=frm1[:], scalar1=1.0)

                # scale = d * (nl*nv) * fr / (4*nl*nv)
                g = sc_tile("g")
                nc.vector.tensor_mul(out=g[:], in0=nl, in1=nv)
                dn = sc_tile("dn")
                nc.scalar.mul(out=dn[:], in_=g[:], mul=4.0)
                rdn = safe_recip(dn, "rdn")
                t1 = sc_tile("t1")
                nc.vector.tensor_mul(out=t1[:], in0=g[:], in1=fr[:])
                t2 = sc_tile("t2")
                nc.vector.tensor_mul(out=t2[:], in0=t1[:], in1=rdn[:])
                scl = sc_tile("scl")
                nc.scalar.mul(out=scl[:], in_=t2[:], mul=d_const)

                # ---- out[t, k] = base_color[k] * scale[t], per tile ----
                for tg in range(group):
                    i = gi * group + tg
                    ot = op_.tile([128, 3 * T], F32, tag="ot", name=f"ot{i}")
                    o3 = ot[:].rearrange("p (t c) -> p t c", c=3)
                    ssl = scl[:, bass.ts(tg, T)]
                    if bc_uniform:
                        nc.scalar.activation(out=o3, in_=ssl.to_broadcast((128, T, 3)),
                                             func=Act.Copy, scale=bc[0])
                    else:
                        for k in range(3):
                            nc.scalar.mul(out=o3[:, :, k], in_=ssl, mul=bc[k])
                    nc.sync.dma_start(out=y[:, bass.ts(i, 3 * T)], in_=ot[:])
    if not nc.is_finalized():
        nc.finalize()
    return nc


def _shard_inputs(inputs: np.ndarray) -> list:
    flat = np.ascontiguousarray(inputs, dtype=np.float32).reshape(N_TOTAL, 9)
    pad = np.ones((S_PAD - S, 9), dtype=np.float32)
    in_maps = []
    for cidx in range(N_CORES):
        sh = np.concatenate([flat[cidx * S:(cidx + 1) * S], pad], axis=0)
        in_maps.append({"x": sh.reshape(128, 9 * ROWS)})
    return in_maps


def _assemble(results: list) -> np.ndarray:
    outs = []
    for cidx in range(N_CORES):
        o = np.asarray(results[cidx]["y"], dtype=np.float32)
        outs.append(o.reshape(S_PAD, 3)[:S])
    return np.concatenate(outs, axis=0)


def run(inputs, base_color, alpha, eta, trace=False, **trace_kwargs):
    del alpha  # unused by MicrofacetBase (d-term stub)
    nc = build_program(float(np.asarray(eta).reshape(-1)[0]),
                       np.asarray(base_color, np.float32).reshape(3))
    in_maps = _shard_inputs(np.asarray(inputs))
    res = run_bass_kernel_spmd(nc, in_maps, list(range(N_CORES)), trace=trace,
                               **trace_kwargs)
    out = _assemble(res.results)
    return out, res


def kernel(inputs, base_color, alpha, eta):
    out, _ = run(inputs, base_color, alpha, eta, trace=False)
    return out


# revision 14
# speedup vs baseline: 3.4365x; 1.6047x over previous
"""Trainium2 Bass kernel for nn_MicrofacetBase (Cook-Torrance microfacet base-class stub).

Per sample i with rows light/normal/view in inputs[i]:
    hv    = light + view
    half  = hv / max(||hv||, EPS)
    c     = view.half
    fr    = cook-torrance fresnel(c, eta)
    d     = 0 (MicrofacetBase stub)
    out   = base_color * (d * nl*nv * fr) / (4 * nl*nv)
          = base_color * d * fr / 4          (nl*nv cancels; fast-math DCE)

Data-parallel across 8 NeuronCores: each core gets 500,000 samples (padded to
128*3912 = 500,736 so every SBUF tile is [128, *]).

Layout: per core the input is reshaped to [128, ROWS, 9] (row-major) so each
partition owns ROWS consecutive samples. Component stage runs per tile of T
rows; the per-sample scalar chain is batched over G=2 tiles (width W=2T) to
amortize per-instruction overhead. The 1/4 factor is folded into the Square
scales of the fresnel combination.

Self-contained: hardcodes shapes/sharding; builds + runs the Bass program via
run_bass_kernel_spmd on cores 0-7.
"""

import numpy as np

from concourse import bass, bacc, mybir
from concourse import tile
from concourse.bass_utils import run_bass_kernel_spmd

F32 = mybir.dt.float32
EPS = 1e-12

N_TOTAL = 4_000_000
N_CORES = 8
S = N_TOTAL // N_CORES          # samples per core = 500,000
ROWS = 3912                     # rows per partition (128*3912 = 500,736 >= S)
S_PAD = 128 * ROWS
TILE_ROWS = 652                 # rows per tile; 6 tiles per core
GROUP = 2                       # tiles per scalar-chain batch
SQRT_EIGHTH = 0.3535533905932738  # sqrt(1/8): folds fr's 0.5 and the 1/4

Alu = mybir.AluOpType
Act = mybir.ActivationFunctionType


def build_program(eta: float, bc: np.ndarray, rows: int = ROWS, tile_rows: int = TILE_ROWS,
                  group: int = GROUP, d_const: float = 0.0) -> bass.Bass:
    """Build the SPMD single-core program. eta/base_color are baked as immediates
    (JIT-style specialization on the scalar params)."""
    assert rows % tile_rows == 0
    n_tiles = rows // tile_rows
    assert n_tiles % group == 0
    n_groups = n_tiles // group
    T = tile_rows
    W = group * T
    K = float(np.float32(np.float32(eta) * np.float32(eta)) - np.float32(1.0))
    bc = [float(v) for v in np.asarray(bc, np.float32)]
    bc_uniform = bc[0] == bc[1] == bc[2]
    always_pos = K > 1e-6  # gg = c^2 + K >= K > 0: where-branch statically true

    nc = bacc.Bacc(None)
    if always_pos:
        # register eta^2-1 as a const AP so Act.Sqrt can take it as bias
        kt = nc.alloc_sbuf_tensor("const-K", [128, 1], F32)
        nc.gpsimd.memset(kt.ap(), K)
        nc.const_aps.aps[(F32, float(K))] = kt.ap()
        nc.all_engine_barrier()
    x = nc.declare_dram_parameter("x", [128, 9 * rows], F32, isOutput=False)
    y = nc.declare_dram_parameter("y", [128, 3 * rows], F32, isOutput=True)

    with tile.TileContext(nc) as tc:
        with tc.tile_pool(name="xp", bufs=2) as xp, \
             tc.tile_pool(name="pp", bufs=2) as pp, \
             tc.tile_pool(name="dp", bufs=2) as dp, \
             tc.tile_pool(name="op", bufs=2) as op_, \
             tc.tile_pool(name="sp", bufs=12) as sp:
            for gi in range(n_groups):
                # ---- component stage: per tile ----
                D = dp.tile([128, 2 * W], F32, tag="D", name=f"D{gi}")
                Dv = D[:].rearrange("p (s w) -> p s w", s=2)
                for tg in range(group):
                    i = gi * group + tg
                    xt = xp.tile([128, 9 * T], F32, tag="xt", name=f"xt{i}")
                    nc.sync.dma_start(out=xt[:], in_=x[:, bass.ts(i, 9 * T)])
                    x3 = xt[:].rearrange("p (t c) -> p t c", c=9)
                    li = x3[:, :, 0:3]
                    nr = x3[:, :, 3:6]
                    vw = x3[:, :, 6:9]

                    # hv = light + view, written over the (unused) normal slots
                    nc.vector.tensor_add(out=nr, in0=li, in1=vw)
                    hv = nr
                    # products for dvh = v.hv (DVE) and s = hv.hv (ACT)
                    pr3 = pp.tile([128, 3 * T], F32, tag="pr", name=f"pr{i}")
                    nc.vector.tensor_mul(out=pr3[:].rearrange("p (t c) -> p t c", c=3),
                                         in0=vw, in1=hv)
                    prs = pp.tile([128, 3 * T], F32, tag="prs", name=f"prs{i}")
                    nc.scalar.square(out=prs[:].rearrange("p (t c) -> p t c", c=3), in_=hv)

                    # reduce into D sections [dvh | s2] at col tg*T
                    nc.vector.reduce_sum(
                        out=Dv[:, 0, bass.ts(tg, T)],
                        in_=pr3[:].rearrange("p (t c) -> p t c", c=3),
                        axis=mybir.AxisListType.X)
                    nc.vector.reduce_sum(
                        out=Dv[:, 1, bass.ts(tg, T)],
                        in_=prs[:].rearrange("p (t c) -> p t c", c=3),
                        axis=mybir.AxisListType.X)

                # ---- per-sample scalar chain: width W ----
                dvh = Dv[:, 0]
                s2 = Dv[:, 1]

                def sc_tile(name):
                    return sp.tile([128, W], F32, tag="sc", name=f"{name}_{gi}")

                # c = dvh / max(sqrt(s), EPS)
                nrm = sc_tile("nrm")
                nc.scalar.sqrt(out=nrm[:], in_=s2)
                nrmx = sc_tile("nrmx")
                nc.vector.tensor_scalar_max(out=nrmx[:], in0=nrm[:], scalar1=EPS)
                inv = sc_tile("inv")
                nc.vector.reciprocal_approx_fast(out=inv[:], in_=nrmx[:])
                c = sc_tile("c")
                nc.vector.tensor_mul(out=c[:], in0=dvh, in1=inv[:])

                # fresnel: gg = c^2 + K
                c2 = sc_tile("c2")
                nc.scalar.square(out=c2[:], in_=c[:])
                if always_pos:
                    gs = sc_tile("gs")
                    nc.scalar.activation(out=gs[:], in_=c2[:], func=Act.Sqrt, bias=K)
                else:
                    gg = sc_tile("gg")
                    nc.vector.tensor_scalar_add(out=gg[:], in0=c2[:], scalar1=K)
                    mask = sc_tile("mask")
                    nc.vector.tensor_scalar(out=mask[:], in0=gg[:], scalar1=0.0,
                                            scalar2=None, op0=Alu.is_gt)
                    ggm = sc_tile("ggm")
                    nc.vector.tensor_scalar_max(out=ggm[:], in0=gg[:], scalar1=EPS)
                    gs = sc_tile("gs")
                    nc.scalar.sqrt(out=gs[:], in_=ggm[:])

                u = sc_tile("u")
                nc.vector.tensor_mul(out=u[:], in0=c[:], in1=gs[:])
                # b-num = c*(gs+c)-1 = (u-1)+c2 ; b-den = c*(gs-c)+1 = (u+1)-c2
                bnum = sc_tile("bnum")
                nc.vector.scalar_tensor_tensor(out=bnum[:], in0=u[:], scalar=-1.0,
                                               in1=c2[:], op0=Alu.add, op1=Alu.add)
                bden = sc_tile("bden")
                nc.vector.scalar_tensor_tensor(out=bden[:], in0=u[:], scalar=1.0,
                                               in1=c2[:], op0=Alu.add, op1=Alu.subtract)
                den1 = sc_tile("den1")
                nc.vector.tensor_add(out=den1[:], in0=gs[:], in1=c[:])
                num1 = sc_tile("num1")
                nc.vector.tensor_sub(out=num1[:], in0=gs[:], in1=c[:])
                r1 = sc_tile("r1")
                nc.vector.reciprocal_approx_fast(out=r1[:], in_=den1[:])
                a = sc_tile("a")
                nc.vector.tensor_mul(out=a[:], in0=num1[:], in1=r1[:])
                r2 = sc_tile("r2")
                nc.vector.reciprocal_approx_fast(out=r2[:], in_=bden[:])
                b = sc_tile("b")
                nc.vector.tensor_mul(out=b[:], in0=bnum[:], in1=r2[:])
                ab = sc_tile("ab")
                nc.vector.tensor_mul(out=ab[:], in0=a[:], in1=b[:])
                # fr/4 = 0.125*a^2*(1+b^2) = (a*sqrt(1/8))^2 + (ab*sqrt(1/8))^2
                sa = sc_tile("sa")
                nc.scalar.activation(out=sa[:], in_=a[:], func=Act.Square, scale=SQRT_EIGHTH)
                sab = sc_tile("sab")
                nc.scalar.activation(out=sab[:], in_=ab[:], func=Act.Square, scale=SQRT_EIGHTH)
                frq = sc_tile("frq")  # = fr/4 (or (fr-?) handled below for generic eta)
                nc.vector.tensor_add(out=frq[:], in0=sa[:], in1=sab[:])
                if always_pos:
                    fr4 = frq  # where(gg>0,...) statically true
                else:
                    # fr/4 = (frq - 0.25)*mask + 0.25
                    frm1 = sc_tile("frm1")
                    nc.vector.scalar_tensor_tensor(out=frm1[:], in0=frq[:], scalar=-0.25,
                                                   in1=mask[:], op0=Alu.add, op1=Alu.mult)
                    fr4 = sc_tile("fr4")
                    nc.vector.tensor_scalar_add(out=fr4[:], in0=frm1[:], scalar1=0.25)

                # scale = d * (nl*nv) * fr / (4*nl*nv) = d * fr/4
                scl = sc_tile("scl")
                nc.scalar.mul(out=scl[:], in_=fr4[:], mul=d_const)

                # ---- out[t, k] = base_color[k] * scale[t], per tile ----
                for tg in range(group):
                    i = gi * group + tg
                    ot = op_.tile([128, 3 * T], F32, tag="ot", name=f"ot{i}")
                    o3 = ot[:].rearrange("p (t c) -> p t c", c=3)
                    ssl = scl[:, bass.ts(tg, T)]
                    if bc_uniform:
                        nc.scalar.activation(out=o3, in_=ssl.to_broadcast((128, T, 3)),
                                             func=Act.Copy, scale=bc[0])
                    else:
                        for k in range(3):
                            nc.scalar.mul(out=o3[:, :, k], in_=ssl, mul=bc[k])
                    nc.sync.dma_start(out=y[:, bass.ts(i, 3 * T)], in_=ot[:])
    if not nc.is_finalized():
        nc.finalize()
    return nc


def _shard_inputs(inputs: np.ndarray) -> list:
    flat = np.ascontiguousarray(inputs, dtype=np.float32).reshape(N_TOTAL, 9)
    pad = np.ones((S_PAD - S, 9), dtype=np.float32)
    in_maps = []
    for cidx in range(N_CORES):
        sh = np.concatenate([flat[cidx * S:(cidx + 1) * S], pad], axis=0)
        in_maps.append({"x": sh.reshape(128, 9 * ROWS)})
    return in_maps


def _assemble(results: list) -> np.ndarray:
    outs = []
    for cidx in range(N_CORES):
        o = np.asarray(results[cidx]["y"], dtype=np.float32)
        outs.append(o.reshape(S_PAD, 3)[:S])
    return np.concatenate(outs, axis=0)


def run(inputs, base_color, alpha, eta, trace=False, **trace_kwargs):
    del alpha  # unused by MicrofacetBase (d-term stub)
    nc = build_program(float(np.asarray(eta).reshape(-1)[0]),
                       np.asarray(base_color, np.float32).reshape(3))
    in_maps = _shard_inputs(np.asarray(inputs))
    res = run_bass_kernel_spmd(nc, in_maps, list(range(N_CORES)), trace=trace,
                               **trace_kwargs)
    out = _assemble(res.results)
    return out, res


def kernel(inputs, base_color, alpha, eta):
    out, _ = run(inputs, base_color, alpha, eta, trace=False)
    return out


# revision 17
# speedup vs baseline: 3.9007x; 1.1351x over previous
"""Trainium2 Bass kernel for nn_MicrofacetBase (Cook-Torrance microfacet base-class stub).

Per sample i with rows light/normal/view in inputs[i]:
    hv    = light + view
    half  = hv / max(||hv||, EPS)
    c     = view.half
    fr    = cook-torrance fresnel(c, eta)
    d     = 0 (MicrofacetBase stub)
    out   = base_color * (d * nl*nv * fr) / (4 * nl*nv)
          = base_color * d * fr / 4          (nl*nv cancels; fast-math DCE)

Data-parallel across 8 NeuronCores: each core gets 500,000 samples (padded to
128*3912 = 500,736 so every SBUF tile is [128, *]).

Layout: per core the input is reshaped to [128, ROWS, 9] (row-major) so each
partition owns ROWS consecutive samples. Component stage runs per tile of T
rows; the per-sample scalar chain is batched over G=2 tiles (width W=2T) to
amortize per-instruction overhead. The 1/4 factor is folded into the Square
scales of the fresnel combination.

Self-contained: hardcodes shapes/sharding; builds + runs the Bass program via
run_bass_kernel_spmd on cores 0-7.
"""

import numpy as np

from concourse import bass, bacc, mybir
from concourse import tile
from concourse.bass_utils import run_bass_kernel_spmd

F32 = mybir.dt.float32
EPS = 1e-12

N_TOTAL = 4_000_000
N_CORES = 8
S = N_TOTAL // N_CORES          # samples per core = 500,000
ROWS = 3912                     # rows per partition (128*3912 = 500,736 >= S)
S_PAD = 128 * ROWS
TILE_ROWS = 652                 # rows per tile; 6 tiles per core
GROUP = 3                       # tiles per scalar-chain batch
SQRT_EIGHTH = 0.3535533905932738  # sqrt(1/8): folds fr's 0.5 and the 1/4

Alu = mybir.AluOpType
Act = mybir.ActivationFunctionType


def build_program(eta: float, bc: np.ndarray, rows: int = ROWS, tile_rows: int = TILE_ROWS,
                  group: int = GROUP, d_const: float = 0.0) -> bass.Bass:
    """Build the SPMD single-core program. eta/base_color are baked as immediates
    (JIT-style specialization on the scalar params)."""
    assert rows % tile_rows == 0
    n_tiles = rows // tile_rows
    assert n_tiles % group == 0
    n_groups = n_tiles // group
    T = tile_rows
    W = group * T
    K = float(np.float32(np.float32(eta) * np.float32(eta)) - np.float32(1.0))
    bc = [float(v) for v in np.asarray(bc, np.float32)]
    bc_uniform = bc[0] == bc[1] == bc[2]
    always_pos = K > 1e-6  # gg = c^2 + K >= K > 0: where-branch statically true

    nc = bacc.Bacc(None)
    if always_pos:
        # register eta^2-1 as a const AP so Act.Sqrt can take it as bias
        kt = nc.alloc_sbuf_tensor("const-K", [128, 1], F32)
        nc.gpsimd.memset(kt.ap(), K)
        nc.const_aps.aps[(F32, float(K))] = kt.ap()
        nc.all_engine_barrier()
    x = nc.declare_dram_parameter("x", [128, 9 * rows], F32, isOutput=False)
    y = nc.declare_dram_parameter("y", [128, 3 * rows], F32, isOutput=True)

    with tile.TileContext(nc) as tc:
        with tc.tile_pool(name="xp", bufs=2) as xp, \
             tc.tile_pool(name="pp", bufs=2) as pp, \
             tc.tile_pool(name="dp", bufs=2) as dp, \
             tc.tile_pool(name="op", bufs=2) as op_, \
             tc.tile_pool(name="sp", bufs=8) as sp:
            for gi in range(n_groups):
                # ---- component stage: per tile ----
                D = dp.tile([128, 2 * W], F32, tag="D", name=f"D{gi}")
                Dv = D[:].rearrange("p (s w) -> p s w", s=2)
                for tg in range(group):
                    i = gi * group + tg
                    xt = xp.tile([128, 9 * T], F32, tag="xt", name=f"xt{i}")
                    nc.sync.dma_start(out=xt[:], in_=x[:, bass.ts(i, 9 * T)])
                    x3 = xt[:].rearrange("p (t c) -> p t c", c=9)
                    li = x3[:, :, 0:3]
                    nr = x3[:, :, 3:6]
                    vw = x3[:, :, 6:9]

                    # hv = light + view, written over the (unused) normal slots
                    nc.vector.tensor_add(out=nr, in0=li, in1=vw)
                    hv = nr
                    # products for dvh = v.hv (DVE) and s = hv.hv (ACT)
                    pr3 = pp.tile([128, 3 * T], F32, tag="pr", name=f"pr{i}")
                    nc.vector.tensor_mul(out=pr3[:].rearrange("p (t c) -> p t c", c=3),
                                         in0=vw, in1=hv)
                    prs = pp.tile([128, 3 * T], F32, tag="prs", name=f"prs{i}")
                    nc.scalar.square(out=prs[:].rearrange("p (t c) -> p t c", c=3), in_=hv)

                    # reduce into D sections [dvh | s2] at col tg*T
                    nc.vector.reduce_sum(
                        out=Dv[:, 0, bass.ts(tg, T)],
                        in_=pr3[:].rearrange("p (t c) -> p t c", c=3),
                        axis=mybir.AxisListType.X)
                    nc.vector.reduce_sum(
                        out=Dv[:, 1, bass.ts(tg, T)],
                        in_=prs[:].rearrange("p (t c) -> p t c", c=3),
                        axis=mybir.AxisListType.X)

                # ---- per-sample scalar chain: width W ----
                dvh = Dv[:, 0]
                s2 = Dv[:, 1]

                def sc_tile(name):
                    return sp.tile([128, W], F32, tag="sc", name=f"{name}_{gi}")

                if always_pos:
                    # Safe squared-denominator fresnel via (gs-c)(gs+c) = K:
                    #   P = gs + c, a = K/P^2, cM+1 = (cK+P)/P = q/P
                    #   fr/4 = K^2/(8 P^4) + K^2 (cP-1)^2 / (8 P^2 q^2)
                    # All reciprocals have positive, clamped inputs.
                    nrm = sc_tile("nrm")
                    nc.scalar.sqrt(out=nrm[:], in_=s2)
                    inv = sc_tile("inv")  # max(nrm,EPS)==nrm for any real data
                    nc.vector.reciprocal_approx_fast(out=inv[:], in_=nrm[:])
                    c = sc_tile("c")
                    nc.vector.tensor_mul(out=c[:], in0=dvh, in1=inv[:])
                    c2 = sc_tile("c2")
                    nc.scalar.square(out=c2[:], in_=c[:])
                    gs = sc_tile("gs")
                    nc.scalar.activation(out=gs[:], in_=c2[:], func=Act.Sqrt, bias=K)
                    P = sc_tile("P")
                    nc.vector.tensor_add(out=P[:], in0=gs[:], in1=c[:])
                    rp = sc_tile("rp")
                    nc.vector.reciprocal_approx_fast(out=rp[:], in_=P[:])
                    t = sc_tile("t")
                    nc.vector.tensor_mul(out=t[:], in0=c[:], in1=P[:])
                    q = sc_tile("q")  # cK + P
                    nc.vector.scalar_tensor_tensor(out=q[:], in0=c[:], scalar=K,
                                                   in1=P[:], op0=Alu.mult, op1=Alu.add)
                    qq = sc_tile("qq")
                    nc.scalar.square(out=qq[:], in_=q[:])
                    qqe = sc_tile("qqe")
                    nc.vector.tensor_scalar_max(out=qqe[:], in0=qq[:], scalar1=1e-30)
                    rqq = sc_tile("rqq")
                    nc.vector.reciprocal_approx_fast(out=rqq[:], in_=qqe[:])
                    rp2 = sc_tile("rp2")
                    nc.scalar.square(out=rp2[:], in_=rp[:])
                    sa = sc_tile("sa")  # = a^2/8 = (K/sqrt8 * rp2)^2
                    nc.scalar.activation(out=sa[:], in_=rp2[:], func=Act.Square,
                                         scale=K * SQRT_EIGHTH)
                    w1 = sc_tile("w1")  # (cP-1)*rp
                    nc.vector.scalar_tensor_tensor(out=w1[:], in0=t[:], scalar=-1.0,
                                                   in1=rp[:], op0=Alu.add, op1=Alu.mult)
                    w1s = sc_tile("w1s")  # K^2 (cP-1)^2 rp^2 / 8
                    nc.scalar.activation(out=w1s[:], in_=w1[:], func=Act.Square,
                                         scale=K * SQRT_EIGHTH)
                    sab = sc_tile("sab")  # = (ab)^2/8
                    nc.vector.tensor_mul(out=sab[:], in0=w1s[:], in1=rqq[:])
                    fr4 = sc_tile("fr4")
                    nc.vector.tensor_add(out=fr4[:], in0=sa[:], in1=sab[:])
                else:
                    # generic-eta path (branch not statically known)
                    nrm = sc_tile("nrm")
                    nc.scalar.sqrt(out=nrm[:], in_=s2)
                    nrmx = sc_tile("nrmx")
                    nc.vector.tensor_scalar_max(out=nrmx[:], in0=nrm[:], scalar1=EPS)
                    inv = sc_tile("inv")
                    nc.vector.reciprocal_approx_fast(out=inv[:], in_=nrmx[:])
                    c = sc_tile("c")
                    nc.vector.tensor_mul(out=c[:], in0=dvh, in1=inv[:])
                    c2 = sc_tile("c2")
                    nc.scalar.square(out=c2[:], in_=c[:])
                    gg = sc_tile("gg")
                    nc.vector.tensor_scalar_add(out=gg[:], in0=c2[:], scalar1=K)
                    mask = sc_tile("mask")
                    nc.vector.tensor_scalar(out=mask[:], in0=gg[:], scalar1=0.0,
                                            scalar2=None, op0=Alu.is_gt)
                    ggm = sc_tile("ggm")
                    nc.vector.tensor_scalar_max(out=ggm[:], in0=gg[:], scalar1=EPS)
                    gs = sc_tile("gs")
                    nc.scalar.sqrt(out=gs[:], in_=ggm[:])
                    u = sc_tile("u")
                    nc.vector.tensor_mul(out=u[:], in0=c[:], in1=gs[:])
                    bnum = sc_tile("bnum")
                    nc.vector.scalar_tensor_tensor(out=bnum[:], in0=u[:], scalar=-1.0,
                                                   in1=c2[:], op0=Alu.add, op1=Alu.add)
                    bden = sc_tile("bden")
                    nc.vector.scalar_tensor_tensor(out=bden[:], in0=u[:], scalar=1.0,
                                                   in1=c2[:], op0=Alu.add, op1=Alu.subtract)
                    den1 = sc_tile("den1")
                    nc.vector.tensor_add(out=den1[:], in0=gs[:], in1=c[:])
                    num1 = sc_tile("num1")
                    nc.vector.tensor_sub(out=num1[:], in0=gs[:], in1=c[:])
                    r1 = sc_tile("r1")
                    nc.vector.reciprocal_approx_fast(out=r1[:], in_=den1[:])
                    a = sc_tile("a")
                    nc.vector.tensor_mul(out=a[:], in0=num1[:], in1=r1[:])
                    r2 = sc_tile("r2")
                    nc.vector.reciprocal_approx_fast(out=r2[:], in_=bden[:])
                    b = sc_tile("b")
                    nc.vector.tensor_mul(out=b[:], in0=bnum[:], in1=r2[:])
                    ab = sc_tile("ab")
                    nc.vector.tensor_mul(out=ab[:], in0=a[:], in1=b[:])
                    sa = sc_tile("sa")
                    nc.scalar.activation(out=sa[:], in_=a[:], func=Act.Square,
                                         scale=SQRT_EIGHTH)
                    sab = sc_tile("sab")
                    nc.scalar.activation(out=sab[:], in_=ab[:], func=Act.Square,
                                         scale=SQRT_EIGHTH)
                    frq = sc_tile("frq")
                    nc.vector.tensor_add(out=frq[:], in0=sa[:], in1=sab[:])
                    # fr/4 = (frq - 0.25)*mask + 0.25
                    frm1 = sc_tile("frm1")
                    nc.vector.scalar_tensor_tensor(out=frm1[:], in0=frq[:], scalar=-0.25,
                                                   in1=mask[:], op0=Alu.add, op1=Alu.mult)
                    fr4 = sc_tile("fr4")
                    nc.vector.tensor_scalar_add(out=fr4[:], in0=frm1[:], scalar1=0.25)

                # ---- out[t, k] = base_color[k] * d * fr/4, per tile ----
                for tg in range(group):
                    i = gi * group + tg
                    ot = op_.tile([128, 3 * T], F32, tag="ot", name=f"ot{i}")
                    o3 = ot[:].rearrange("p (t c) -> p t c", c=3)
                    ssl = fr4[:, bass.ts(tg, T)]
                    if bc_uniform:
                        nc.scalar.activation(out=o3, in_=ssl.to_broadcast((128, T, 3)),
                                             func=Act.Copy, scale=bc[0] * d_const)
                    else:
                        for k in range(3):
                            nc.scalar.mul(out=o3[:, :, k], in_=ssl, mul=bc[k] * d_const)
                    nc.sync.dma_start(out=y[:, bass.ts(i, 3 * T)], in_=ot[:])
    if not nc.is_finalized():
        nc.finalize()
    return nc


def _shard_inputs(inputs: np.ndarray) -> list:
    flat = np.ascontiguousarray(inputs, dtype=np.float32).reshape(N_TOTAL, 9)
    pad = np.ones((S_PAD - S, 9), dtype=np.float32)
    in_maps = []
    for cidx in range(N_CORES):
        sh = np.concatenate([flat[cidx * S:(cidx + 1) * S], pad], axis=0)
        in_maps.append({"x": sh.reshape(128, 9 * ROWS)})
    return in_maps


def _assemble(results: list) -> np.ndarray:
    outs = []
    for cidx in range(N_CORES):
        o = np.asarray(results[cidx]["y"], dtype=np.float32)
        outs.append(o.reshape(S_PAD, 3)[:S])
    return np.concatenate(outs, axis=0)


def run(inputs, base_color, alpha, eta, trace=False, **trace_kwargs):
    del alpha  # unused by MicrofacetBase (d-term stub)
    nc = build_program(float(np.asarray(eta).reshape(-1)[0]),
                       np.asarray(base_color, np.float32).reshape(3))
    in_maps = _shard_inputs(np.asarray(inputs))
    res = run_bass_kernel_spmd(nc, in_maps, list(range(N_CORES)), trace=trace,
                               **trace_kwargs)
    out = _assemble(res.results)
    return out, res


def kernel(inputs, base_color, alpha, eta):
    out, _ = run(inputs, base_color, alpha, eta, trace=False)
    return out
